# revision 3
# baseline (speedup 1.0000x reference)
"""AttentionPool Trainium2 Bass kernel (v2: fp8 DoubleRow gates + interleaved
phases).

Computes, for h:[N,512] f32, sorted batch_vec:[N] int, gate-MLP weights
W1/b1/W2/b2:
    gate  = gelu(h @ W1 + b1) @ W2 + b2            (erf gelu)
    alpha = segment_softmax(gate, batch_vec)       (1024 segments)
    out   = segment_sum(alpha[:,None] * h)         -> [1024, 512]

Sharding: data-parallel over graphs. Core c owns graphs [128c, 128c+128)
and the contiguous node range covering them (batch_vec sorted => segments
never straddle cores).

Gate matmul (mm1) runs in fp8 e4m3 with DoubleRow perf mode using an
error-compensated 3-term split (h = h8 + hr, W1 = W18 + W1r):
    z = (h8 + hr) @ W18 + h8 @ W1r
which is 6 DoubleRow matmuls per (supertile, dout-chunk) = 1536 PE cycles
vs 2048 for f16, with max rel err ~1e-2 (budget 2e-2). Pooling stays f16
(fp8 selection weights fail the error budget). Phase C (pool) is emitted
interleaved with phase A so the hp DMA stream and pool matmuls overlap
the gate computation; exp runs in 3 batches to bound ACT table switches.
The softmax max-subtraction is skipped: gates are O(1) so exp is safe in
fp32, and the result is mathematically identical.
"""

import os
from contextlib import ExitStack, nullcontext

import numpy as np

import concourse.bass as bass
import concourse.mybir as mybir
from concourse import bacc
import concourse.tile as tile
from concourse.bass_utils import run_bass_kernel_spmd

F32 = mybir.dt.float32
F16 = mybir.dt.float16
F8 = mybir.dt.float8e4

N_NODES = 100000
H = 512
NUM_GRAPHS = 1024
N_CORES = 8
G = NUM_GRAPHS // N_CORES  # graphs per core = 128
NP_DEFAULT = 12800         # padded nodes per core (25 supertiles of 512)

_NPDT = {"f32": np.float32}
import ml_dtypes
_NPDT["bf16"] = ml_dtypes.bfloat16
_NPDT["f16"] = np.float16
NP_F8 = mybir.dt.np(F8)

MM1 = os.environ.get("AP_MM1", "f8x3")   # "f8x3" | "f16"
HP_BUFS = int(os.environ.get("AP_HP_BUFS", "12"))
HT_BUFS = int(os.environ.get("AP_HT_BUFS", "4"))
# supertile indices after whose mm2 each exp batch runs (must end with S-1)
EXP_AT = tuple(int(x) for x in os.environ.get("AP_EXP_AT", "7,16,24").split(","))


def _build(np_pad: int, mm1_mode: str = None, reps: int = 1, ablate: str = ""):
    """Build the per-core Bass program (SPMD: same program, per-core data)."""
    if mm1_mode is None:
        mm1_mode = MM1
    T = np_pad // 128          # 128-node tiles
    S = np_pad // 512          # 512-node supertiles
    KC = H // 128              # contraction chunks = 4

    nc = bacc.Bacc("TRN2", target_bir_lowering=False, debug=False)

    if mm1_mode == "f8x3":
        # [S, p, which(h8/hr), kpair, j, n] fp8 — 4KB/partition/supertile
        hx_d = nc.dram_tensor("hx", [S, 128, 2, 2, 2, 512], F8,
                              kind="ExternalInput")
        # [which, kpair, p, j, dout] fp8
        w1x_d = nc.dram_tensor("w1x", [2, 2, 128, 2, 512], F8,
                               kind="ExternalInput")
    else:
        # [S, p, k, n] f16 — 4KB/partition/supertile
        hx_d = nc.dram_tensor("hx", [S, 128, KC, 512], F16,
                              kind="ExternalInput")
        w1x_d = nc.dram_tensor("w1x", [KC, 128, 512], F16,
                               kind="ExternalInput")
    hp_d = nc.dram_tensor("hp", [S, 128, 4, 512], F16, kind="ExternalInput")
    b1_d = nc.dram_tensor("b1v", [128, KC], F32, kind="ExternalInput")
    w2_d = nc.dram_tensor("W2v", [128, KC * 2], F16, kind="ExternalInput")
    b2_d = nc.dram_tensor("b2t", [128, 1], F32, kind="ExternalInput")
    bv_d = nc.dram_tensor("bvrel", [128, T], F32, kind="ExternalInput")
    io_d = nc.dram_tensor("iota", [128, 128], F32, kind="ExternalInput")
    out_d = nc.dram_tensor("out", [G, H], F32, kind="ExternalOutput")

    with tile.TileContext(nc) as tc, ExitStack() as ctx:
        consts = ctx.enter_context(tc.tile_pool(name="consts", bufs=1))
        ht_pool = ctx.enter_context(tc.tile_pool(name="ht", bufs=HT_BUFS))
        a1_pool = ctx.enter_context(tc.tile_pool(name="a1", bufs=8))
        hp_pool = ctx.enter_context(tc.tile_pool(name="hp", bufs=HP_BUFS))
        ms_pool = ctx.enter_context(tc.tile_pool(name="ms", bufs=4))
        small = ctx.enter_context(tc.tile_pool(name="small", bufs=2))
        psz = ctx.enter_context(tc.tile_pool(name="psz", bufs=4, space="PSUM"))
        psg = ctx.enter_context(tc.tile_pool(name="psg", bufs=2, space="PSUM"))
        psp = ctx.enter_context(tc.tile_pool(name="psp", bufs=1, space="PSUM"))
        psd = ctx.enter_context(tc.tile_pool(name="psd", bufs=1, space="PSUM"))

        # ---- constants ----
        if mm1_mode == "f8x3":
            w1_sb = []  # [which][kpair] -> tile [128, 2, 512]
            for w in range(2):
                row = []
                for i in range(2):
                    t = consts.tile([128, 2, 512], F8, tag=f"w1_{w}{i}")
                    nc.sync.dma_start(out=t, in_=w1x_d.ap()[w, i])
                    row.append(t)
                w1_sb.append(row)
        else:
            w1_sb = []
            for k in range(KC):
                t = consts.tile([128, 512], F16, tag=f"w1_{k}")
                nc.sync.dma_start(out=t, in_=w1x_d.ap()[k])
                w1_sb.append(t)
        b1_sb = consts.tile([128, KC], F32, tag="b1")
        nc.sync.dma_start(out=b1_sb, in_=b1_d.ap())
        w2_sb = consts.tile([128, KC * 2], F16, tag="w2")
        nc.sync.dma_start(out=w2_sb, in_=w2_d.ap())
        b2_sb = consts.tile([128, 1], F32, tag="b2")
        nc.sync.dma_start(out=b2_sb, in_=b2_d.ap())
        io_sb = consts.tile([128, 128], F32, tag="iota")
        nc.sync.dma_start(out=io_sb, in_=io_d.ap())
        bv_sb = consts.tile([128, T], F32, tag="bv")
        nc.sync.dma_start(out=bv_sb, in_=bv_d.ap())
        ones_sb = consts.tile([128, 2], F16, tag="ones")
        nc.vector.memset(ones_sb, 1.0)
        gate_sb = consts.tile([128, T], F32, tag="gate")
        e_sb = consts.tile([128, T], F32, tag="e")

        gelu = mybir.ActivationFunctionType.Gelu
        expf = mybir.ActivationFunctionType.Exp

        loop_cm = tc.For_i(0, reps, 1) if reps > 1 else nullcontext()
        with loop_cm:
            if ablate == "noA":
                nc.vector.memset(gate_sb, 0.125)
                nc.scalar.activation(out=e_sb, in_=gate_sb, func=expf,
                                     bias=b2_sb[:, 0:1], scale=1.0)
            if ablate not in ("noC", "dmaonly"):
                pp = psp.tile([128, H], F32, tag="pp")
                pd = psd.tile([128, 2], F32, tag="pd")

            hp_tiles = {}
            exp_done = 0            # tiles whose e is computed
            pool_done = 0           # supertiles pooled so far
            n_exp = 0

            def do_pool(s_lo, s_hi):
                """Emit ms + pool matmuls for supertiles [s_lo, s_hi)."""
                for s4 in range(s_lo, s_hi):
                    hpb = hp_tiles.pop(s4)
                    for j in range(4):
                        t = s4 * 4 + j
                        ms = ms_pool.tile([128, 128], F16, tag="ms")
                        # ms[n, g] = (iota[n,g] == bvrel[n]) * e[n]
                        nc.vector.tensor_scalar(
                            out=ms, in0=io_sb,
                            scalar1=bv_sb[:, t:t + 1],
                            scalar2=e_sb[:, t:t + 1],
                            op0=mybir.AluOpType.is_equal,
                            op1=mybir.AluOpType.mult)
                        nc.tensor.matmul(out=pp, lhsT=ms, rhs=hpb[:, j, :],
                                         start=(t == 0), stop=(t == T - 1))
                        nc.tensor.matmul(out=pd, lhsT=ms, rhs=ones_sb,
                                         start=(t == 0), stop=(t == T - 1))

            for s in range(S):
                # -- DMA issues (both streams interleaved in queue order) --
                if ablate not in ("noA",):
                    if mm1_mode == "f8x3":
                        htb = ht_pool.tile([128, 2, 2, 2, 512], F8, tag="ht")
                    else:
                        htb = ht_pool.tile([128, KC, 512], F16, tag="ht")
                    nc.sync.dma_start(out=htb, in_=hx_d.ap()[s])
                if ablate not in ("noC",):
                    hpb = hp_pool.tile([128, 4, 512], F16, tag="hp")
                    nc.sync.dma_start(out=hpb, in_=hp_d.ap()[s])
                    hp_tiles[s] = hpb

                if ablate == "dmaonly":
                    continue

                # -- phase A compute for supertile s --
                if ablate != "noA":
                    a1s = []
                    for d in range(KC):
                        pz = psz.tile([128, 512], F32, tag="pz")
                        if mm1_mode == "f8x3":
                            mm = 0
                            for wsel, xsel in ((0, 0), (0, 1), (1, 0)):
                                # term: W1[wsel] contraction with h[xsel]
                                # (0,0)=h8@W18 (0,1)=hr@W18 (1,0)=h8@W1r
                                for i in range(2):
                                    nc.tensor.matmul(
                                        out=pz,
                                        lhsT=w1_sb[wsel][i][
                                            :, :, d * 128:(d + 1) * 128],
                                        rhs=htb[:, xsel, i, :, :],
                                        start=(mm == 0), stop=(mm == 5),
                                        perf_mode=mybir.MatmulPerfMode.DoubleRow)
                                    mm += 1
                        else:
                            for k in range(KC):
                                nc.tensor.matmul(
                                    out=pz,
                                    lhsT=w1_sb[k][:, d * 128:(d + 1) * 128],
                                    rhs=htb[:, k, :],
                                    start=(k == 0), stop=(k == KC - 1))
                        a1 = a1_pool.tile([128, 512], F16, tag="a1")
                        nc.scalar.activation(out=a1, in_=pz, func=gelu,
                                             bias=b1_sb[:, d:d + 1], scale=1.0)
                        a1s.append(a1)
                    if ablate == "nogate":
                        nc.vector.memset(gate_sb[:, s * 4:(s + 1) * 4], 0.125)
                    else:
                        pg = psg.tile([128, 2 * KC], F32, tag="pg")
                        for nch in range(4):
                            for d in range(KC):
                                nc.tensor.matmul(
                                    out=pg[:, 2 * nch:2 * nch + 2],
                                    lhsT=a1s[d][:, nch * 128:(nch + 1) * 128],
                                    rhs=w2_sb[:, 2 * d:2 * d + 2],
                                    start=(d == 0), stop=(d == KC - 1))
                        nc.vector.tensor_copy(
                            out=gate_sb[:, s * 4:(s + 1) * 4],
                            in_=pg[:, 0:2 * KC:2])

                # -- exp batch + interleaved pool emission --
                if ablate in ("", "noC") and s in EXP_AT:
                    t_hi = (s + 1) * 4
                    nc.scalar.activation(
                        out=e_sb[:, exp_done:t_hi],
                        in_=gate_sb[:, exp_done:t_hi],
                        func=expf, bias=b2_sb[:, 0:1], scale=1.0)
                    exp_done = t_hi
                    n_exp += 1
                    if ablate == "" and s < S - 1:
                        do_pool(pool_done, s + 1)
                        pool_done = s + 1

            # -- tail pools --
            if ablate == "nogate":
                nc.scalar.activation(out=e_sb, in_=gate_sb, func=expf,
                                     bias=b2_sb[:, 0:1], scale=1.0)
            if ablate not in ("noC", "dmaonly"):
                do_pool(pool_done, S)

            osb = small.tile([128, H], F32, tag="osb")
            if ablate in ("noC", "dmaonly"):
                nc.vector.memset(osb, 0.0)
            else:
                dcl = small.tile([128, 1], F32, tag="dcl")
                nc.vector.tensor_scalar(out=dcl, in0=pd[:, 0:1], scalar1=1e-35,
                                        scalar2=None, op0=mybir.AluOpType.max)
                rec = small.tile([128, 1], F32, tag="rec")
                nc.vector.reciprocal(out=rec, in_=dcl)
                nc.vector.tensor_scalar(out=osb, in0=pp, scalar1=rec[:, 0:1],
                                        scalar2=None, op0=mybir.AluOpType.mult)
            nc.sync.dma_start(out=out_d.ap(), in_=osb)

    nc.compile()
    return nc


_prog_cache: dict = {}


def _get_prog(np_pad: int):
    key = (np_pad, MM1, EXP_AT)
    if key not in _prog_cache:
        _prog_cache[key] = _build(np_pad)
    return _prog_cache[key]


def _prep_in_maps(h, bv, W1, b1, W2, b2, np_pad, mm1_mode=None):
    """Shard + pad inputs per core; returns list of per-core input dicts."""
    if mm1_mode is None:
        mm1_mode = MM1
    T = np_pad // 128
    S = np_pad // 512
    bounds = np.searchsorted(bv, np.arange(0, NUM_GRAPHS + 1, G))

    if mm1_mode == "f8x3":
        W18 = W1.astype(NP_F8)
        W1r = (W1 - W18.astype(np.float32)).astype(NP_F8)
        # [which, kpair, p, j, dout]; k = kpair*256 + j*128 + p
        w1x = np.stack([
            np.ascontiguousarray(
                w.reshape(2, 2, 128, H).transpose(0, 2, 1, 3))
            for w in (W18, W1r)])
    else:
        # [k, p, dout]
        w1x = np.ascontiguousarray(
            W1.astype(np.float16).reshape(KC_ := 4, 128, H))
    b1v = np.ascontiguousarray(b1.astype(np.float32).reshape(4, 128).T)
    w2v = np.zeros((128, 8), np.float16)
    w2v[:, 0::2] = W2[:, 0].astype(np.float16).reshape(4, 128).T
    b2t = np.full((128, 1), np.float32(b2.reshape(-1)[0]), np.float32)
    iota = np.ascontiguousarray(
        np.tile(np.arange(128, dtype=np.float32), (128, 1)))

    in_maps = []
    for c in range(N_CORES):
        n0, n1 = int(bounds[c]), int(bounds[c + 1])
        cnt = n1 - n0
        hpad = np.zeros((np_pad, H), np.float32)
        hpad[:cnt] = h[n0:n1]
        # hp: [S, p, j, d]; node = s*512 + j*128 + p
        hp = np.ascontiguousarray(
            hpad.astype(np.float16).reshape(S, 4, 128, H)
            .transpose(0, 2, 1, 3))
        if mm1_mode == "f8x3":
            h8 = hpad.astype(NP_F8)
            hr = (hpad - h8.astype(np.float32)).astype(NP_F8)
            # [S, p, which, kpair, j, n]; hidden k = kpair*256 + j*128 + p
            hx = np.stack([
                x.reshape(S, 512, 2, 2, 128).transpose(0, 4, 2, 3, 1)
                for x in (h8, hr)], axis=2)
            hx = np.ascontiguousarray(hx)
        else:
            # [S, p, k, n]
            hx = np.ascontiguousarray(
                hpad.astype(np.float16).reshape(S, 512, 4, 128)
                .transpose(0, 3, 2, 1))
        bvrel = np.full(np_pad, -1.0, np.float32)
        bvrel[:cnt] = bv[n0:n1].astype(np.float32) - c * G
        bvrel = np.ascontiguousarray(bvrel.reshape(T, 128).T)
        in_maps.append({
            "hx": hx,
            "hp": hp,
            "w1x": w1x,
            "b1v": b1v,
            "W2v": w2v,
            "b2t": b2t,
            "bvrel": bvrel,
            "iota": iota,
        })
    return in_maps


def kernel(**inputs) -> np.ndarray:
    h = np.ascontiguousarray(np.asarray(inputs["h"], dtype=np.float32))
    bv = np.asarray(inputs["batch_vec"]).astype(np.int64)
    W1 = np.asarray(inputs["W1"], dtype=np.float32)
    b1 = np.asarray(inputs["b1"], dtype=np.float32)
    W2 = np.asarray(inputs["W2"], dtype=np.float32)
    b2 = np.asarray(inputs["b2"], dtype=np.float32)

    bounds = np.searchsorted(bv, np.arange(0, NUM_GRAPHS + 1, G))
    max_cnt = int(np.diff(bounds).max())
    np_pad = NP_DEFAULT
    if max_cnt > np_pad:  # fallback for unexpected distributions
        np_pad = ((max_cnt + 511) // 512) * 512

    nc = _get_prog(np_pad)
    in_maps = _prep_in_maps(h, bv, W1, b1, W2, b2, np_pad)
    trace = bool(int(os.environ.get("AP_TRACE", "0")))
    res = run_bass_kernel_spmd(nc, in_maps, list(range(N_CORES)), trace=trace)
    global last_results
    last_results = res
    out = np.concatenate([res.results[c]["out"] for c in range(N_CORES)],
                         axis=0).astype(np.float32)
    return out


last_results = None


# revision 10
# speedup vs baseline: 1.3891x; 1.3891x over previous
"""AttentionPool Trainium2 Bass kernel (v4: flipped gate layout, no mm2).

Computes, for h:[N,512] f32, sorted batch_vec:[N] int, gate-MLP weights
W1/b1/W2/b2:
    gate  = gelu(h @ W1 + b1) @ W2 + b2            (erf gelu)
    alpha = segment_softmax(gate, batch_vec)       (1024 segments)
    out   = segment_sum(alpha[:,None] * h)         -> [1024, 512]

Sharding: data-parallel over graphs. Core c owns graphs [128c, 128c+128)
and the contiguous node range covering them (batch_vec sorted => segments
never straddle cores).

Structure ("flip" mode, requires b1 == 0 which holds for this module):
  mm1 computes z in [node, dout] layout per 128-node tile:
      z_t = (hT_k)^T @ W1_k  accumulated over k-chunks  -> psum [128n, 512d]
  gelu on ACT -> a1 [128n, 512d] f16, then the gate dot-product runs on
  the DVE as a fused multiply+reduce against a partition-replicated W2:
      gate[n] = reduce_add(a1[n,:] * W2rep[n,:]) + b2
  eliminating the narrow a1@W2 tensor matmuls, their PSUM bank, and the
  gate-layout copy. exp runs in a few batches on ACT (table switches cost
  1.3us, so batches are few); pool matmuls + the hp DMA stream trickle
  through the tensor stream between supertiles once their exp batch is
  done. DMA layouts are host-packed so every DMA reads 4KB contiguous per
  partition. The softmax max-subtraction is skipped: gates are O(1) so
  exp is safe in fp32, and the result is mathematically identical.
"""

import os
from contextlib import ExitStack, nullcontext

import numpy as np

import concourse.bass as bass
import concourse.mybir as mybir
from concourse import bacc
import concourse.tile as tile
from concourse.bass_utils import run_bass_kernel_spmd

F32 = mybir.dt.float32
F16 = mybir.dt.float16

N_NODES = 100000
H = 512
NUM_GRAPHS = 1024
N_CORES = 8
G = NUM_GRAPHS // N_CORES  # graphs per core = 128
NP_DEFAULT = 12800         # padded nodes per core (25 supertiles of 512)

MODE = os.environ.get("AP_MODE", "flip")   # "flip" | "mm2"
HP_BUFS = int(os.environ.get("AP_HP_BUFS", "12"))
HT_BUFS = int(os.environ.get("AP_HT_BUFS", "4"))
# supertile indices after which each exp batch runs (must end with S-1)
EXP_AT = tuple(int(x) for x in os.environ.get("AP_EXP_AT", "11,20,24").split(","))
# max pool supertiles trickled into the tensor stream per phase-A step
POOL_RATE = int(os.environ.get("AP_POOL_RATE", "3"))
# engine queue for the hp DMA stream ("sync" shares the hx queue)
HPQ = os.environ.get("AP_HPQ", "sync")


def _build(np_pad: int, mode: str = None, reps: int = 1, ablate: str = ""):
    """Build the per-core Bass program (SPMD: same program, per-core data)."""
    if mode is None:
        mode = MODE
    T = np_pad // 128          # 128-node tiles
    S = np_pad // 512          # 512-node supertiles
    KC = H // 128              # contraction chunks = 4

    nc = bacc.Bacc("TRN2", target_bir_lowering=False, debug=False)

    # [S, p, k, n] f16 - 4KB/partition/supertile
    hx_d = nc.dram_tensor("hx", [S, 128, KC, 512], F16, kind="ExternalInput")
    w1x_d = nc.dram_tensor("w1x", [KC, 128, 512], F16, kind="ExternalInput")
    hp_d = nc.dram_tensor("hp", [S, 128, 4, 512], F16, kind="ExternalInput")
    b2_d = nc.dram_tensor("b2t", [128, 1], F32, kind="ExternalInput")
    bv_d = nc.dram_tensor("bvrel", [128, T], F32, kind="ExternalInput")
    io_d = nc.dram_tensor("iota", [128, 128], F32, kind="ExternalInput")
    if mode == "flip":
        w2r_d = nc.dram_tensor("w2rep", [128, H], F16, kind="ExternalInput")
    else:
        b1_d = nc.dram_tensor("b1v", [128, KC], F32, kind="ExternalInput")
        w2_d = nc.dram_tensor("W2v", [128, KC * 2], F16, kind="ExternalInput")
    out_d = nc.dram_tensor("out", [G, H], F32, kind="ExternalOutput")

    with tile.TileContext(nc) as tc, ExitStack() as ctx:
        consts = ctx.enter_context(tc.tile_pool(name="consts", bufs=1))
        ht_pool = ctx.enter_context(tc.tile_pool(name="ht", bufs=HT_BUFS))
        a1_pool = ctx.enter_context(tc.tile_pool(name="a1", bufs=8))
        hp_pool = ctx.enter_context(tc.tile_pool(name="hp", bufs=HP_BUFS))
        ms_pool = ctx.enter_context(tc.tile_pool(name="ms", bufs=4))
        small = ctx.enter_context(tc.tile_pool(name="small", bufs=2))
        nzb = 6 if mode == "flip" else 4
        psz = ctx.enter_context(tc.tile_pool(name="psz", bufs=nzb, space="PSUM"))
        if mode != "flip":
            psg = ctx.enter_context(tc.tile_pool(name="psg", bufs=2,
                                                 space="PSUM"))
        psp = ctx.enter_context(tc.tile_pool(name="psp", bufs=1, space="PSUM"))
        psd = ctx.enter_context(tc.tile_pool(name="psd", bufs=1, space="PSUM"))

        # ---- constants ----
        w1_sb = []
        for k in range(KC):
            t = consts.tile([128, 512], F16, tag=f"w1_{k}")
            nc.sync.dma_start(out=t, in_=w1x_d.ap()[k])
            w1_sb.append(t)
        if mode == "flip":
            w2r_sb = consts.tile([128, H], F16, tag="w2rep")
            nc.sync.dma_start(out=w2r_sb, in_=w2r_d.ap())
            gs_pool = ctx.enter_context(tc.tile_pool(name="gs", bufs=2))
        else:
            b1_sb = consts.tile([128, KC], F32, tag="b1")
            nc.sync.dma_start(out=b1_sb, in_=b1_d.ap())
            w2_sb = consts.tile([128, KC * 2], F16, tag="w2")
            nc.sync.dma_start(out=w2_sb, in_=w2_d.ap())
        b2_sb = consts.tile([128, 1], F32, tag="b2")
        nc.sync.dma_start(out=b2_sb, in_=b2_d.ap())
        io_sb = consts.tile([128, 128], F32, tag="iota")
        nc.sync.dma_start(out=io_sb, in_=io_d.ap())
        bv_sb = consts.tile([128, T], F32, tag="bv")
        nc.sync.dma_start(out=bv_sb, in_=bv_d.ap())
        ones_sb = consts.tile([128, 2], F16, tag="ones")
        nc.vector.memset(ones_sb, 1.0)
        gate_sb = consts.tile([128, T], F32, tag="gate")
        e_sb = consts.tile([128, T], F32, tag="e")

        gelu = mybir.ActivationFunctionType.Gelu
        expf = mybir.ActivationFunctionType.Exp

        loop_cm = tc.For_i(0, reps, 1) if reps > 1 else nullcontext()
        with loop_cm:
            if ablate == "noA":
                nc.vector.memset(gate_sb, 0.125)
                nc.scalar.activation(out=e_sb, in_=gate_sb, func=expf,
                                     bias=b2_sb[:, 0:1], scale=1.0)
            if ablate not in ("noC", "dmaonly"):
                pp = psp.tile([128, H], F32, tag="pp")
                pd = psd.tile([128, 2], F32, tag="pd")

            hp_tiles = {}
            exp_done = 0            # tiles whose e is computed
            pool_done = 0           # supertiles pooled so far

            def do_pool(s_lo, s_hi):
                """Emit ms + pool matmuls for supertiles [s_lo, s_hi)."""
                for s4 in range(s_lo, s_hi):
                    hpb = hp_tiles.pop(s4)
                    for j in range(4):
                        t = s4 * 4 + j
                        ms = ms_pool.tile([128, 128], F16, tag="ms")
                        # ms[n, g] = (iota[n,g] == bvrel[n]) * e[n]
                        nc.vector.tensor_scalar(
                            out=ms, in0=io_sb,
                            scalar1=bv_sb[:, t:t + 1],
                            scalar2=e_sb[:, t:t + 1],
                            op0=mybir.AluOpType.is_equal,
                            op1=mybir.AluOpType.mult)
                        nc.tensor.matmul(out=pp, lhsT=ms, rhs=hpb[:, j, :],
                                         start=(t == 0), stop=(t == T - 1))
                        nc.tensor.matmul(out=pd, lhsT=ms, rhs=ones_sb,
                                         start=(t == 0), stop=(t == T - 1))

            for s in range(S):
                # -- DMA issues (both streams interleaved in queue order) --
                if ablate not in ("noA",):
                    htb = ht_pool.tile([128, KC, 512], F16, tag="ht")
                    nc.sync.dma_start(out=htb, in_=hx_d.ap()[s])
                if ablate not in ("noC",):
                    hpb = hp_pool.tile([128, 4, 512], F16, tag="hp")
                    getattr(nc, HPQ).dma_start(out=hpb, in_=hp_d.ap()[s])
                    hp_tiles[s] = hpb

                if ablate == "dmaonly":
                    continue

                # -- phase A compute for supertile s --
                if ablate != "noA":
                    if mode == "flip":
                        for nch in range(4):
                            tt = s * 4 + nch
                            pz = psz.tile([128, H], F32, tag="pz")
                            for k in range(KC):
                                nc.tensor.matmul(
                                    out=pz,
                                    lhsT=htb[:, k, nch * 128:(nch + 1) * 128],
                                    rhs=w1_sb[k],
                                    start=(k == 0), stop=(k == KC - 1))
                            a1 = a1_pool.tile([128, H], F16, tag="a1")
                            nc.scalar.activation(out=a1, in_=pz, func=gelu,
                                                 scale=1.0)
                            if ablate == "nogate":
                                if nch == 0:
                                    nc.vector.memset(
                                        gate_sb[:, s * 4:(s + 1) * 4], 0.125)
                            else:
                                scr = gs_pool.tile([128, H], F16, tag="scr")
                                nc.vector.scalar_tensor_tensor(
                                    out=scr, in0=a1, scalar=1.0, in1=w2r_sb,
                                    op0=mybir.AluOpType.mult,
                                    op1=mybir.AluOpType.mult,
                                    accum_out=gate_sb[:, tt:tt + 1])
                    else:
                        a1s = []
                        for d in range(KC):
                            pz = psz.tile([128, H], F32, tag="pz")
                            for k in range(KC):
                                nc.tensor.matmul(
                                    out=pz,
                                    lhsT=w1_sb[k][:, d * 128:(d + 1) * 128],
                                    rhs=htb[:, k, :],
                                    start=(k == 0), stop=(k == KC - 1))
                            a1 = a1_pool.tile([128, H], F16, tag="a1")
                            nc.scalar.activation(out=a1, in_=pz, func=gelu,
                                                 bias=b1_sb[:, d:d + 1],
                                                 scale=1.0)
                            a1s.append(a1)
                        if ablate == "nogate":
                            nc.vector.memset(
                                gate_sb[:, s * 4:(s + 1) * 4], 0.125)
                        else:
                            pg = psg.tile([128, 2 * KC], F32, tag="pg")
                            for nch in range(4):
                                for d in range(KC):
                                    nc.tensor.matmul(
                                        out=pg[:, 2 * nch:2 * nch + 2],
                                        lhsT=a1s[d][
                                            :, nch * 128:(nch + 1) * 128],
                                        rhs=w2_sb[:, 2 * d:2 * d + 2],
                                        start=(d == 0), stop=(d == KC - 1))
                            nc.vector.tensor_copy(
                                out=gate_sb[:, s * 4:(s + 1) * 4],
                                in_=pg[:, 0:2 * KC:2])

                # -- exp batch --
                if ablate in ("", "noC") and s in EXP_AT:
                    t_hi = (s + 1) * 4
                    nc.scalar.activation(
                        out=e_sb[:, exp_done:t_hi],
                        in_=gate_sb[:, exp_done:t_hi],
                        func=expf, bias=b2_sb[:, 0:1], scale=1.0)
                    exp_done = t_hi
                # -- trickle pool work for exp-ready supertiles --
                if ablate == "":
                    lim = min(exp_done // 4, s)  # strictly-behind supertiles
                    hi = min(pool_done + POOL_RATE, lim)
                    if hi > pool_done:
                        do_pool(pool_done, hi)
                        pool_done = hi

            # -- tail pools --
            if ablate == "nogate":
                nc.scalar.activation(out=e_sb, in_=gate_sb, func=expf,
                                     bias=0.0, scale=1.0)
            if ablate not in ("noC", "dmaonly"):
                do_pool(pool_done, S)

            osb = small.tile([128, H], F32, tag="osb")
            if ablate in ("noC", "dmaonly"):
                nc.vector.memset(osb, 0.0)
            else:
                dcl = small.tile([128, 1], F32, tag="dcl")
                nc.vector.tensor_scalar(out=dcl, in0=pd[:, 0:1], scalar1=1e-35,
                                        scalar2=None, op0=mybir.AluOpType.max)
                rec = small.tile([128, 1], F32, tag="rec")
                nc.vector.reciprocal(out=rec, in_=dcl)
                nc.vector.tensor_scalar(out=osb, in0=pp, scalar1=rec[:, 0:1],
                                        scalar2=None, op0=mybir.AluOpType.mult)
            nc.sync.dma_start(out=out_d.ap(), in_=osb)

    nc.compile()
    return nc


_prog_cache: dict = {}


def _get_prog(np_pad: int, mode: str = None):
    if mode is None:
        mode = MODE
    key = (np_pad, mode, EXP_AT, POOL_RATE)
    if key not in _prog_cache:
        _prog_cache[key] = _build(np_pad, mode=mode)
    return _prog_cache[key]


def _prep_in_maps(h, bv, W1, b1, W2, b2, np_pad, mode=None):
    """Shard + pad inputs per core; returns list of per-core input dicts."""
    if mode is None:
        mode = MODE
    T = np_pad // 128
    S = np_pad // 512
    bounds = np.searchsorted(bv, np.arange(0, NUM_GRAPHS + 1, G))

    # [k, p, dout]
    w1x = np.ascontiguousarray(W1.astype(np.float16).reshape(4, 128, H))
    b2t = np.full((128, 1), np.float32(b2.reshape(-1)[0]), np.float32)
    iota = np.ascontiguousarray(
        np.tile(np.arange(128, dtype=np.float32), (128, 1)))
    common = {"w1x": w1x, "b2t": b2t, "iota": iota}
    if mode == "flip":
        common["w2rep"] = np.ascontiguousarray(
            np.tile(W2[:, 0].astype(np.float16), (128, 1)))
    else:
        common["b1v"] = np.ascontiguousarray(
            b1.astype(np.float32).reshape(4, 128).T)
        w2v = np.zeros((128, 8), np.float16)
        w2v[:, 0::2] = W2[:, 0].astype(np.float16).reshape(4, 128).T
        common["W2v"] = w2v

    in_maps = []
    for c in range(N_CORES):
        n0, n1 = int(bounds[c]), int(bounds[c + 1])
        cnt = n1 - n0
        hpad = np.zeros((np_pad, H), np.float32)
        hpad[:cnt] = h[n0:n1]
        # hp: [S, p, j, d]; node = s*512 + j*128 + p
        hp = np.ascontiguousarray(
            hpad.astype(np.float16).reshape(S, 4, 128, H)
            .transpose(0, 2, 1, 3))
        # hx: [S, p, k, n]; hidden = k*128 + p, node = s*512 + n
        hx = np.ascontiguousarray(
            hpad.astype(np.float16).reshape(S, 512, 4, 128)
            .transpose(0, 3, 2, 1))
        bvrel = np.full(np_pad, -1.0, np.float32)
        bvrel[:cnt] = bv[n0:n1].astype(np.float32) - c * G
        bvrel = np.ascontiguousarray(bvrel.reshape(T, 128).T)
        in_maps.append({"hx": hx, "hp": hp, "bvrel": bvrel, **common})
    return in_maps


def kernel(**inputs) -> np.ndarray:
    h = np.ascontiguousarray(np.asarray(inputs["h"], dtype=np.float32))
    bv = np.asarray(inputs["batch_vec"]).astype(np.int64)
    W1 = np.asarray(inputs["W1"], dtype=np.float32)
    b1 = np.asarray(inputs["b1"], dtype=np.float32)
    W2 = np.asarray(inputs["W2"], dtype=np.float32)
    b2 = np.asarray(inputs["b2"], dtype=np.float32)

    bounds = np.searchsorted(bv, np.arange(0, NUM_GRAPHS + 1, G))
    max_cnt = int(np.diff(bounds).max())
    np_pad = NP_DEFAULT
    if max_cnt > np_pad:  # fallback for unexpected distributions
        np_pad = ((max_cnt + 511) // 512) * 512

    # flip mode folds b1 away (it is zero for this module); fall back to
    # the mm2 structure for nonzero b1.
    mode = MODE
    if mode == "flip" and np.any(b1 != 0):
        mode = "mm2"

    nc = _get_prog(np_pad, mode)
    in_maps = _prep_in_maps(h, bv, W1, b1, W2, b2, np_pad, mode)
    trace = bool(int(os.environ.get("AP_TRACE", "0")))
    res = run_bass_kernel_spmd(nc, in_maps, list(range(N_CORES)), trace=trace)
    global last_results
    last_results = res
    out = np.concatenate([res.results[c]["out"] for c in range(N_CORES)],
                         axis=0).astype(np.float32)
    return out


last_results = None
